# revision 26
# baseline (speedup 1.0000x reference)
"""GAT (2-layer, 8-head) Bass kernel for 8 Trainium2 NeuronCores.

Sharding: rows (nodes) split 512/core; x and params replicated.
Layer-1 attention per head in transposed layout (partition=j, free=i);
softmax row sums fold into the aggregation matmul via an augmented
column.

Max-form attention: with rho_i = exp(-(1-a) f1_i), t_j = exp((1-a) f2_j):
    m_ij * max(rho_i, t_j) * G2_j = exp(-f1_i) * m_ij * exp(lrelu(f1_i+f2_j))
and the per-row factor exp(-f1_i) cancels in the softmax normalize.
Each head-tile needs ONE fused DVE op q = (rho_b max t_j) * mask
(scalar_tensor_tensor) and ONE matmul against XmAll = G2-scaled Wh_aug
(built during stage 1, fused into the Wh pass; aug column = G2).

Schedule: wave 1 (4 heads) interleaves its STT+matmul stream into the
stage-1 Wh/F loop (PE and DVE both busy); wave 2 runs 3 more max-form
heads plus optionally A-form heads on the otherwise-idle Scalar engine
(exact exp path, rescaled by exp(-a f2) so it shares the same XmAll
stationary operand).  Layer 2 uses the same max-form after a small
AllGather of per-core [512, 41] Wh2 shards.  elu's "-1" is absorbed
algebraically (colsum correction + log_softmax shift invariance).
Row-sum reciprocals are batched into one [8,512] op; the elu tail runs
once on the packed [128,4,512] concat layout with PE-built normalizer
rows.
"""
import sys

sys.path.insert(0, "/opt/trn_rl_repo")

import numpy as np
import ml_dtypes

import concourse.bass as bass
import concourse.bacc as bacc
import concourse.tile as tile
import concourse.mybir as mybir
from concourse.bass_utils import run_bass_kernel_spmd

F32 = mybir.dt.float32
BF16 = mybir.dt.bfloat16
AF = mybir.ActivationFunctionType
ALU = mybir.AluOpType
AX = mybir.AxisListType

NCORES = 8
N = 4096
FIN = 256
HID = 64
H = 8
NC = 41          # classes
ROWS = N // NCORES   # 512 rows per core
JT = N // 128        # 32 j tiles
IT = ROWS // 128     # 4 i tiles of my rows
AUG = HID + 1        # 65
AUG2 = NC + 1        # 42
ALPHA = 0.2

WAVE1 = (0, 1, 2, 3)
A_COUNT = 2          # heads of wave 2 on the Scalar (exp) path

_CACHED_NC = None


def _build(trace_sim=False, reps=1, ablate=()):
    nc = bacc.Bacc("TRN2", target_bir_lowering=False, debug=False,
                   num_devices=NCORES)
    d = {}
    def dram_in(name, shape, dt=F32):
        d[name] = nc.dram_tensor(name, list(shape), dt, kind="ExternalInput").ap()
        return d[name]

    xT = dram_in("xT", [128, 2, N])
    xrT = dram_in("xrT", [128, 2, ROWS])
    wcat = dram_in("wcat", [128, 2, H * HID])
    wa = dram_in("wa", [128, 2, 2 * H])
    wout = dram_in("wout", [128, 4, NC])
    woa1 = dram_in("woa1", [128, 4])
    a2b = dram_in("a2b", [128, NC])
    csum = dram_in("csum", [128, NC])
    ident = dram_in("ident", [128, NC])
    consts = dram_in("consts", [128, 8])
    sel = dram_in("sel", [8, 4, 128])
    maskT = dram_in("maskT", [128, JT, ROWS], BF16)
    out = nc.dram_tensor("out", [ROWS, NC], F32, kind="ExternalOutput").ap()

    a_count = A_COUNT
    if "A0" in ablate:
        a_count = 0
    if "A2" in ablate:
        a_count = 2
    wave2_m = tuple(range(4, 8 - a_count))
    wave2_a = tuple(range(8 - a_count, 8))

    with tile.TileContext(nc, trace_sim=trace_sim) as tc:
        with (
            tc.tile_pool(name="dram", bufs=1, space="DRAM") as dpool,
            tc.tile_pool(name="const", bufs=1) as cp,
            tc.tile_pool(name="big", bufs=1) as bigp,
            tc.tile_pool(name="rbp", bufs=8) as rbp,
            tc.tile_pool(name="qp", bufs=6) as qp,
            tc.tile_pool(name="work", bufs=4) as wp,
            tc.tile_pool(name="work2", bufs=2) as wp2,
            tc.tile_pool(name="head1", bufs=1) as hp1,
            tc.tile_pool(name="psA", bufs=2, space="PSUM") as psA,
            tc.tile_pool(name="psB", bufs=4, space="PSUM") as psB,
            tc.tile_pool(name="psS", bufs=2, space="PSUM") as psS,
        ):
            # ---------------- stage 0: loads ----------------
            xrT_sb = cp.tile([128, 2, ROWS], F32)
            nc.sync.dma_start(out=xrT_sb[:], in_=xrT[:])
            wcat_sb = cp.tile([128, 2, H * HID], F32)
            nc.sync.dma_start(out=wcat_sb[:], in_=wcat[:])
            wa_sb = cp.tile([128, 2, 2 * H], F32)
            nc.sync.dma_start(out=wa_sb[:], in_=wa[:])
            wout_sb = cp.tile([128, 4, NC], F32)
            nc.sync.dma_start(out=wout_sb[:], in_=wout[:])
            woa1_sb = cp.tile([128, 4], F32)
            nc.sync.dma_start(out=woa1_sb[:], in_=woa1[:])
            a2b_sb = cp.tile([128, NC], F32)
            nc.sync.dma_start(out=a2b_sb[:], in_=a2b[:])
            csum_sb = cp.tile([128, NC], F32)
            nc.sync.dma_start(out=csum_sb[:], in_=csum[:])
            ident_sb = cp.tile([128, NC], F32)
            nc.sync.dma_start(out=ident_sb[:], in_=ident[:])
            consts_sb = cp.tile([128, 8], F32)
            nc.sync.dma_start(out=consts_sb[:], in_=consts[:])
            sel_sb = cp.tile([8, 4, 128], F32)
            nc.sync.dma_start(out=sel_sb[:], in_=sel[:])
            mask_sb = bigp.tile([128, JT, ROWS], BF16)
            for mc in range(4):
                nc.sync.dma_start(out=mask_sb[:, mc * 8:(mc + 1) * 8, :],
                                  in_=maskT[:, mc * 8:(mc + 1) * 8, :])

            def body():
                # ---------------- prologue: rho for my rows ----------------
                pfmy = psS.tile([2 * H, ROWS], F32, tag="s")
                for kt in range(2):
                    nc.tensor.matmul(pfmy[:], wa_sb[:, kt, :], xrT_sb[:, kt, :],
                                     start=(kt == 0), stop=(kt == 1))
                rho_bf = cp.tile([2 * H, ROWS], BF16)
                nc.scalar.activation(rho_bf[:], pfmy[:], AF.Exp, scale=-(1.0 - ALPHA))
                fmy_bf = cp.tile([2 * H, ROWS], BF16)
                nc.scalar.copy(fmy_bf[:], pfmy[:])
                # per-head broadcasts: rho for max-form heads, f1 for A-form
                rb = {}
                for h in range(H):
                    src = fmy_bf if h in wave2_a else rho_bf
                    rs = hp1.tile([1, ROWS], BF16, tag="f1s")
                    nc.sync.dma_start(out=rs[:], in_=src[2 * h:2 * h + 1, :])
                    rb[h] = rbp.tile([128, ROWS], BF16, tag="rb", name=f"rb{h}")
                    nc.gpsimd.partition_broadcast(rb[h][:], rs[:])

                # ---------------- stage 1 + wave-1 heads ----------------
                XmAll = bigp.tile([128, H, JT, AUG], BF16)
                F_sb = cp.tile([128, 2 * H, JT], F32)
                t_sb = cp.tile([128, 2 * H, JT], F32)
                pa = {h: psB.tile([AUG, ROWS], F32, tag="pp", name=f"pa{h}")
                      for h in WAVE1}
                for it in range(JT):
                    xt_t = wp.tile([128, 2, 128], F32, tag="xt")
                    nc.sync.dma_start(out=xt_t[:], in_=xT[:, :, it * 128:(it + 1) * 128])
                    pwh = psA.tile([128, H * HID], F32, tag="pa")
                    for kt in range(2):
                        nc.tensor.matmul(pwh[:], xt_t[:, kt, :],
                                         wcat_sb[:, kt, :], start=(kt == 0), stop=(kt == 1))
                    pf = psS.tile([128, 2 * H], F32, tag="s")
                    for kt in range(2):
                        nc.tensor.matmul(pf[:], xt_t[:, kt, :],
                                         wa_sb[:, kt, :], start=(kt == 0), stop=(kt == 1))
                    nc.scalar.copy(F_sb[:, :, it], pf[:])
                    nc.scalar.activation(t_sb[:, :, it], pf[:], AF.Exp,
                                         scale=(1.0 - ALPHA))
                    Gt = wp.tile([128, 2 * H], BF16, tag="Gt")
                    nc.scalar.activation(Gt[:], pf[:], AF.Exp, scale=ALPHA)
                    g2 = Gt.rearrange("p (h two) -> p h two", two=2)[:, :, 1:2]
                    nc.vector.tensor_tensor(
                        XmAll[:, :, it, 0:HID],
                        pwh.rearrange("p (h d) -> p h d", h=H),
                        g2.broadcast_to([128, H, HID]),
                        op=ALU.mult)
                    nc.scalar.copy(XmAll[:, :, it, HID:AUG], g2)
                    for h in WAVE1:
                        q = qp.tile([128, ROWS], BF16, tag="q")
                        nc.vector.scalar_tensor_tensor(
                            q[:], rb[h][:], t_sb[:, 2 * h + 1, it:it + 1],
                            mask_sb[:, it, :], op0=ALU.max, op1=ALU.mult)
                        nc.tensor.matmul(pa[h][:], XmAll[:, h, it, :], q[:],
                                         start=(it == 0), stop=(it == JT - 1))

                xcU = bigp.tile([128, 4, ROWS], F32)     # un-normalized heads
                rows8x = bigp.tile([1, H, ROWS], F32)    # per-head row sums

                def head_out(pah, h):
                    nc.scalar.copy(
                        xcU[(h % 2) * HID:(h % 2) * HID + HID, h // 2, :], pah[0:HID, :])
                    nc.scalar.copy(rows8x[0:1, h, :], pah[HID:AUG, :])

                for h in WAVE1:
                    head_out(pa[h], h)

                # ---------------- wave-2 heads ----------------
                if wave2_a:
                    nAF = cp.tile([128, 2 * H, JT], F32)
                    nc.vector.tensor_scalar(nAF[:], F_sb[:], -ALPHA, None, op0=ALU.mult)
                pa2w = {h: psB.tile([AUG, ROWS], F32, tag="pp", name=f"paw{h}")
                        for h in wave2_m + wave2_a}
                for g in range(JT // 4):
                    for h in wave2_a:
                        j0 = g * 4
                        pt4 = wp2.tile([128, 4, ROWS], BF16, tag="pt")
                        for qq in range(4):
                            jt = j0 + qq
                            et = wp.tile([128, ROWS], F32, tag="et")
                            nc.scalar.activation(et[:], rb[h][:], AF.Prelu,
                                                 bias=F_sb[:, 2 * h + 1, jt:jt + 1],
                                                 alpha=ALPHA)
                            # exp(lrelu(s) - a f2)  -> shares G2-scaled XmAll
                            nc.scalar.activation(pt4[:, qq, :], et[:], AF.Exp,
                                                 bias=nAF[:, 2 * h + 1, jt:jt + 1])
                        pmt = wp2.tile([128, 4, ROWS], BF16, tag="pmt")
                        nc.vector.tensor_tensor(pmt[:], pt4[:],
                                                mask_sb[:, j0:j0 + 4, :], op=ALU.mult)
                        for qq in range(4):
                            jt = j0 + qq
                            nc.tensor.matmul(pa2w[h][:], XmAll[:, h, jt, :],
                                             pmt[:, qq, :],
                                             start=(jt == 0), stop=(jt == JT - 1))
                    for qq in range(4):
                        jt = g * 4 + qq
                        for h in wave2_m:
                            q = qp.tile([128, ROWS], BF16, tag="q")
                            nc.vector.scalar_tensor_tensor(
                                q[:], rb[h][:], t_sb[:, 2 * h + 1, jt:jt + 1],
                                mask_sb[:, jt, :], op0=ALU.max, op1=ALU.mult)
                            nc.tensor.matmul(pa2w[h][:], XmAll[:, h, jt, :], q[:],
                                             start=(jt == 0), stop=(jt == JT - 1))
                for h in wave2_m + wave2_a:
                    head_out(pa2w[h], h)

                # ---------------- batched tail: normalize + elu' ----------------
                rows8 = bigp.tile([8, ROWS], F32)
                nc.sync.dma_start(out=rows8[:], in_=rows8x[:])
                rr8 = bigp.tile([8, ROWS], F32)
                nc.vector.reciprocal(rr8[:], rows8[:])
                xcT = bigp.tile([128, 4, ROWS], F32)
                hn = bigp.tile([128, 4, ROWS], F32)
                for k in range(4):
                    prb = psA.tile([128, ROWS], F32, tag="pa")
                    nc.tensor.matmul(prb[:], sel_sb[:, k, :], rr8[:],
                                     start=True, stop=True)
                    nc.vector.tensor_tensor(hn[:, k, :], xcU[:, k, :], prb[:],
                                            op=ALU.mult)
                tm = bigp.tile([128, 4, ROWS], F32, tag="tm4")
                nc.vector.tensor_scalar(tm[:], hn[:], 0.0, None, op0=ALU.min)
                nc.scalar.activation(xcU[:], tm[:], AF.Exp)
                nc.vector.scalar_tensor_tensor(xcT[:], hn[:], 0.0, xcU[:],
                                               op0=ALU.max, op1=ALU.add)

                # ---------------- stage 3: Wh2 + gather ----------------
                wh2_sb = cp.tile([128, 4, NC], BF16)
                for it in range(IT):
                    pw2 = psS.tile([128, NC], F32, tag="s")
                    for kt in range(4):
                        nc.tensor.matmul(pw2[:], xcT[:, kt, it * 128:(it + 1) * 128],
                                         wout_sb[:, kt, :], start=(kt == 0), stop=(kt == 3))
                    nc.vector.scalar_tensor_tensor(wh2_sb[:, it, :], pw2[:], 0.0,
                                                   csum_sb[:], op0=ALU.add, op1=ALU.subtract)
                ag_in = dpool.tile([128, 4, NC], BF16)
                nc.gpsimd.dma_start(ag_in[:], wh2_sb[:])
                ag_out = dpool.tile([NCORES, 128, 4, NC], BF16)
                nc.gpsimd.collective_compute(
                    "AllGather", ALU.bypass,
                    replica_groups=[list(range(NCORES))],
                    ins=[ag_in.opt()], outs=[ag_out.opt()],
                )
                # gathered rows: core r, it, p -> global row r*512 + it*128 + p
                wh2f = cp.tile([128, JT, AUG2], BF16)
                nc.gpsimd.memset(wh2f[:, :, NC:AUG2], 1.0)
                for r in range(NCORES):
                    nc.sync.dma_start(out=wh2f[:, r * 4:(r + 1) * 4, 0:NC],
                                      in_=ag_out[r])

                # f1 for my rows (layer 2): [1, 512] psum
                pf1o = psS.tile([1, ROWS], F32, tag="s")
                for kt in range(4):
                    nc.tensor.matmul(pf1o[:], woa1_sb[:, kt:kt + 1],
                                     xcT[:, kt, :], start=(kt == 0), stop=(kt == 3))
                R1o_bf = cp.tile([1, ROWS], BF16)
                nc.scalar.activation(R1o_bf[:], pf1o[:], AF.Exp, scale=-(1.0 - ALPHA),
                                     bias=consts_sb[0:1, 2:3])

                # f2 for all nodes (layer 2)
                f2o = cp.tile([128, JT], F32)
                t41b = bigp.tile([128, JT, NC], F32, tag="tm4")
                a2b3 = a2b_sb[:].rearrange("p (o c) -> p o c", o=1)
                nc.vector.tensor_tensor(t41b[:], wh2f[:, :, 0:NC],
                    a2b3.broadcast_to([128, JT, NC]), op=ALU.mult)
                nc.vector.reduce_sum(f2o[:].rearrange("p (k o) -> p k o", o=1),
                                     t41b[:], axis=AX.X)
                t2o = cp.tile([128, JT], F32)
                nc.scalar.activation(t2o[:], f2o[:], AF.Exp, scale=(1.0 - ALPHA))
                G2o = cp.tile([128, JT], F32)
                nc.scalar.activation(G2o[:], f2o[:], AF.Exp, scale=ALPHA)

                # ---------------- layer-2 attention (max-form) ----------------
                rb2i = rbp.tile([128, ROWS], BF16, tag="rb")
                nc.gpsimd.partition_broadcast(rb2i[:], R1o_bf[:])
                Xm2 = hp1.tile([128, JT, AUG2], BF16, tag="Xm2")
                G2o3 = G2o[:].rearrange("p (k o) -> p k o", o=1)
                nc.vector.tensor_tensor(Xm2[:], wh2f[:],
                    G2o3.broadcast_to([128, JT, AUG2]), op=ALU.mult)
                pa2 = psB.tile([AUG2, ROWS], F32, tag="pp")
                for jt in range(JT):
                    q = qp.tile([128, ROWS], BF16, tag="q")
                    nc.vector.scalar_tensor_tensor(
                        q[:], rb2i[:], t2o[:, jt:jt + 1],
                        mask_sb[:, jt, :], op0=ALU.max, op1=ALU.mult)
                    nc.tensor.matmul(pa2[:], Xm2[:, jt, :], q[:],
                                     start=(jt == 0), stop=(jt == JT - 1))
                # normalize + elu'
                hs2 = hp1.tile([AUG2, ROWS], F32, tag="hs2")
                nc.vector.tensor_copy(hs2[:], pa2[:])
                srow2 = hp1.tile([1, ROWS], F32, tag="r1s")
                nc.sync.dma_start(out=srow2[:], in_=hs2[NC:AUG2, :])
                rr2 = hp1.tile([1, ROWS], F32, tag="rr")
                nc.vector.reciprocal(rr2[:], srow2[:])
                rb2 = hp1.tile([128, ROWS], F32, tag="rb2")
                nc.gpsimd.partition_broadcast(rb2[:], rr2[:])
                zn = hp1.tile([NC, ROWS], F32, tag="hn")
                nc.vector.tensor_tensor(zn[:], hs2[0:NC, :], rb2[0:NC, :], op=ALU.mult)
                tm2 = hp1.tile([NC, ROWS], F32, tag="tm")
                nc.vector.tensor_scalar(tm2[:], zn[:], 0.0, None, op0=ALU.min)
                te2 = hp1.tile([NC, ROWS], F32, tag="te")
                nc.scalar.activation(te2[:], tm2[:], AF.Exp)
                zel = hp1.tile([NC, ROWS], F32, tag="zel")
                nc.vector.scalar_tensor_tensor(zel[:], zn[:], 0.0, te2[:],
                                               op0=ALU.max, op1=ALU.add)

                # ---------------- stage 4: log_softmax + out ----------------
                outr = out.rearrange("(t p) c -> p t c", p=128)
                for it in range(IT):
                    ztp = psS.tile([128, NC], F32, tag="s")
                    nc.tensor.transpose(ztp[:], zel[:, it * 128:(it + 1) * 128],
                                        ident_sb[0:NC, 0:NC])
                    zmax = wp.tile([128, 1], F32, tag="zmax")
                    nc.vector.reduce_max(zmax[:], ztp[:], axis=AX.X)
                    nzmax = wp.tile([128, 1], F32, tag="nzmax")
                    nc.vector.tensor_scalar(nzmax[:], zmax[:], -1.0, None, op0=ALU.mult)
                    zsum = wp.tile([128, 1], F32, tag="zsum")
                    zs = wp.tile([128, NC], F32, tag="zs")
                    nc.scalar.activation(zs[:], ztp[:], AF.Exp, bias=nzmax[:],
                                         accum_out=zsum[:])
                    lse = wp.tile([128, 1], F32, tag="lse")
                    nc.scalar.activation(lse[:], zsum[:], AF.Ln)
                    bo = wp.tile([128, 1], F32, tag="bo")
                    nc.vector.scalar_tensor_tensor(bo[:], zmax[:], -1.0, lse[:],
                                                   op0=ALU.mult, op1=ALU.subtract)
                    zf = wp.tile([128, NC], F32, tag="zf")
                    nc.scalar.activation(zf[:], ztp[:], AF.Identity, bias=bo[:])
                    nc.sync.dma_start(out=outr[:, it, :], in_=zf[:])

            for _rep in range(reps):
                body()

    nc.compile()
    return nc


def _host_prep(x, adj, W, a, W_out, a_out):
    bf16 = ml_dtypes.bfloat16
    f32 = np.float32
    x = np.asarray(x, f32)
    W = np.asarray(W, f32)
    a = np.asarray(a, f32)
    W_out = np.asarray(W_out, f32)
    a_out = np.asarray(a_out, f32)

    def pk(arr, kt):  # [kt*128, M] -> [128, kt, M]
        return np.ascontiguousarray(
            arr.reshape(kt, 128, *arr.shape[1:]).transpose(1, 0, *range(2, arr.ndim + 1)))

    xT = pk(np.ascontiguousarray(x.T), 2)                      # [128,2,4096]
    wcat = pk(np.concatenate(list(W), axis=1), 2)              # [128,2,512]
    WA = np.zeros((FIN, 2 * H), f32)
    for h in range(H):
        WA[:, 2 * h] = W[h] @ a[h, :HID]
        WA[:, 2 * h + 1] = W[h] @ a[h, HID:]
    wa = pk(WA, 2)
    wout = pk(W_out, 4)                                        # [128,4,41]
    Woa1 = W_out @ a_out[:NC]                                  # [512]
    woa1 = np.ascontiguousarray(Woa1.reshape(4, 128).T)        # [128,4]
    s = float(Woa1.sum())
    a2b = np.ascontiguousarray(np.broadcast_to(a_out[NC:], (128, NC)))
    csum = np.ascontiguousarray(np.broadcast_to(W_out.sum(0), (128, NC)))
    ident = np.eye(128, NC, dtype=f32)
    consts = np.zeros((128, 8), f32)
    consts[:, 0] = -s
    consts[:, 1] = -ALPHA * s
    consts[:, 2] = (1.0 - ALPHA) * s
    sel = np.zeros((8, 4, 128), f32)
    for h in range(H):
        sel[h, h // 2, (h % 2) * HID:(h % 2) * HID + HID] = 1.0

    shared = dict(xT=xT, wcat=wcat, wa=wa, wout=wout, woa1=woa1, a2b=a2b,
                  csum=csum, ident=ident, consts=consts, sel=sel)
    in_maps = []
    for c in range(NCORES):
        rows = slice(c * ROWS, (c + 1) * ROWS)
        mT = (np.asarray(adj[rows]).T > 0).astype(bf16)        # [4096, 512]
        mT = np.ascontiguousarray(mT.reshape(JT, 128, ROWS).transpose(1, 0, 2))
        xr = pk(np.ascontiguousarray(x[rows].T), 2)            # [128,2,512]
        in_maps.append({**shared, "maskT": mT, "xrT": xr})
    return in_maps


def kernel(x, adj, W, a, W_out, a_out):
    global _CACHED_NC
    if _CACHED_NC is None:
        _CACHED_NC = _build()
    in_maps = _host_prep(x, adj, W, a, W_out, a_out)
    res = run_bass_kernel_spmd(_CACHED_NC, in_maps, list(range(NCORES)))
    out = np.concatenate([res.results[c]["out"] for c in range(NCORES)], axis=0)
    return out.astype(np.float32)


# revision 31
# speedup vs baseline: 1.0340x; 1.0340x over previous
"""GAT (2-layer, 8-head) Bass kernel for 8 Trainium2 NeuronCores.

Sharding: rows (nodes) split 512/core; x and params replicated.
Layer-1 attention per head in transposed layout (partition=j, free=i);
softmax row sums fold into the aggregation matmul via an augmented
column.

Max-form attention: with rho_i = exp(-(1-a) f1_i), t_j = exp((1-a) f2_j):
    m_ij * max(rho_i, t_j) * G2_j = exp(-f1_i) * m_ij * exp(lrelu(f1_i+f2_j))
and the per-row factor exp(-f1_i) cancels in the softmax normalize.
Each head-tile needs ONE fused DVE op q = (rho_b max t_j) * mask
(scalar_tensor_tensor) and ONE matmul against XmAll = G2-scaled Wh_aug
(built during stage 1, fused into the Wh pass; aug column = G2).

Schedule: wave 1 (4 heads) interleaves its STT+matmul stream into the
stage-1 Wh/F loop (PE and DVE both busy); wave 2 runs 3 more max-form
heads plus optionally A-form heads on the otherwise-idle Scalar engine
(exact exp path, rescaled by exp(-a f2) so it shares the same XmAll
stationary operand).  Layer 2 uses the same max-form after a small
AllGather of per-core [512, 41] Wh2 shards.  elu's "-1" is absorbed
algebraically (colsum correction + log_softmax shift invariance).
Row-sum reciprocals are batched into one [8,512] op; the elu tail runs
once on the packed [128,4,512] concat layout with PE-built normalizer
rows.
"""
import sys

sys.path.insert(0, "/opt/trn_rl_repo")

import numpy as np
import ml_dtypes

import concourse.bass as bass
import concourse.bacc as bacc
import concourse.tile as tile
import concourse.mybir as mybir
from concourse.bass_utils import run_bass_kernel_spmd

F32 = mybir.dt.float32
BF16 = mybir.dt.bfloat16
AF = mybir.ActivationFunctionType
ALU = mybir.AluOpType
AX = mybir.AxisListType

NCORES = 8
N = 4096
FIN = 256
HID = 64
H = 8
NC = 41          # classes
ROWS = N // NCORES   # 512 rows per core
JT = N // 128        # 32 j tiles
IT = ROWS // 128     # 4 i tiles of my rows
AUG = HID + 1        # 65
AUG2 = NC + 1        # 42
ALPHA = 0.2

WAVE1 = (0, 1, 2, 3)
A_COUNT = 1          # heads of wave 2 on the Scalar (exp) path

_CACHED_NC = None


def _build(trace_sim=False, reps=1, ablate=()):
    nc = bacc.Bacc("TRN2", target_bir_lowering=False, debug=False,
                   num_devices=NCORES)
    d = {}
    def dram_in(name, shape, dt=F32):
        d[name] = nc.dram_tensor(name, list(shape), dt, kind="ExternalInput").ap()
        return d[name]

    xT = dram_in("xT", [128, 2, N])
    xrT = dram_in("xrT", [128, 2, ROWS])
    wcat = dram_in("wcat", [128, 2, H * HID])
    wa = dram_in("wa", [128, 2, 2 * H])
    wout = dram_in("wout", [128, 4, NC])
    woa1 = dram_in("woa1", [128, 4])
    a2b = dram_in("a2b", [128, NC])
    csum = dram_in("csum", [128, NC])
    ident = dram_in("ident", [128, NC])
    consts = dram_in("consts", [128, 8])
    sel = dram_in("sel", [8, 4, 128])
    maskT = dram_in("maskT", [128, JT, ROWS], BF16)
    out = nc.dram_tensor("out", [ROWS, NC], F32, kind="ExternalOutput").ap()

    a_count = A_COUNT
    if "A0" in ablate:
        a_count = 0
    if "A2" in ablate:
        a_count = 2
    wave2_m = tuple(range(4, 8 - a_count))
    wave2_a = tuple(range(8 - a_count, 8))

    with tile.TileContext(nc, trace_sim=trace_sim) as tc:
        with (
            tc.tile_pool(name="dram", bufs=1, space="DRAM") as dpool,
            tc.tile_pool(name="const", bufs=1) as cp,
            tc.tile_pool(name="big", bufs=1) as bigp,
            tc.tile_pool(name="rbp", bufs=8) as rbp,
            tc.tile_pool(name="qp", bufs=6) as qp,
            tc.tile_pool(name="work", bufs=4) as wp,
            tc.tile_pool(name="work2", bufs=2) as wp2,
            tc.tile_pool(name="head1", bufs=1) as hp1,
            tc.tile_pool(name="psA", bufs=2, space="PSUM") as psA,
            tc.tile_pool(name="psB", bufs=4, space="PSUM") as psB,
            tc.tile_pool(name="psS", bufs=2, space="PSUM") as psS,
        ):
            # ---------------- stage 0: loads ----------------
            xrT_sb = cp.tile([128, 2, ROWS], F32)
            nc.sync.dma_start(out=xrT_sb[:], in_=xrT[:])
            wcat_sb = cp.tile([128, 2, H * HID], F32)
            nc.sync.dma_start(out=wcat_sb[:], in_=wcat[:])
            wa_sb = cp.tile([128, 2, 2 * H], F32)
            nc.sync.dma_start(out=wa_sb[:], in_=wa[:])
            wout_sb = cp.tile([128, 4, NC], F32)
            nc.sync.dma_start(out=wout_sb[:], in_=wout[:])
            woa1_sb = cp.tile([128, 4], F32)
            nc.sync.dma_start(out=woa1_sb[:], in_=woa1[:])
            a2b_sb = cp.tile([128, NC], F32)
            nc.sync.dma_start(out=a2b_sb[:], in_=a2b[:])
            csum_sb = cp.tile([128, NC], F32)
            nc.sync.dma_start(out=csum_sb[:], in_=csum[:])
            ident_sb = cp.tile([128, NC], F32)
            nc.sync.dma_start(out=ident_sb[:], in_=ident[:])
            consts_sb = cp.tile([128, 8], F32)
            nc.sync.dma_start(out=consts_sb[:], in_=consts[:])
            sel_sb = cp.tile([8, 4, 128], F32)
            nc.sync.dma_start(out=sel_sb[:], in_=sel[:])
            mask_sb = bigp.tile([128, JT, ROWS], BF16)
            for mc in range(4):
                nc.scalar.dma_start(out=mask_sb[:, mc * 8:(mc + 1) * 8, :],
                                   in_=maskT[:, mc * 8:(mc + 1) * 8, :])

            def body():
                # ---------------- prologue: rho for my rows ----------------
                pfmy = psS.tile([2 * H, ROWS], F32, tag="s")
                for kt in range(2):
                    nc.tensor.matmul(pfmy[:], wa_sb[:, kt, :], xrT_sb[:, kt, :],
                                     start=(kt == 0), stop=(kt == 1))
                rho_bf = cp.tile([2 * H, ROWS], BF16)
                nc.scalar.activation(rho_bf[:], pfmy[:], AF.Exp, scale=-(1.0 - ALPHA))
                fmy_bf = cp.tile([2 * H, ROWS], BF16)
                nc.scalar.copy(fmy_bf[:], pfmy[:])
                # per-head broadcasts: rho for max-form heads, f1 for A-form
                rb = {}
                for h in range(H):
                    src = fmy_bf if h in wave2_a else rho_bf
                    rs = hp1.tile([1, ROWS], BF16, tag="f1s")
                    nc.sync.dma_start(out=rs[:], in_=src[2 * h:2 * h + 1, :])
                    rb[h] = rbp.tile([128, ROWS], BF16, tag="rb", name=f"rb{h}")
                    nc.gpsimd.partition_broadcast(rb[h][:], rs[:])

                # ---------------- stage 1 + wave-1 heads ----------------
                XmAll = bigp.tile([128, H, JT, AUG], BF16)
                F_sb = cp.tile([128, 2 * H, JT], F32)
                t_sb = cp.tile([128, 2 * H, JT], F32)
                pa = {h: psB.tile([AUG, ROWS], F32, tag="pp", name=f"pa{h}")
                      for h in WAVE1}
                for it in range(JT):
                    xt_t = wp.tile([128, 2, 128], F32, tag="xt")
                    nc.sync.dma_start(out=xt_t[:], in_=xT[:, :, it * 128:(it + 1) * 128])
                    pwh = psA.tile([128, H * HID], F32, tag="pa")
                    for kt in range(2):
                        nc.tensor.matmul(pwh[:], xt_t[:, kt, :],
                                         wcat_sb[:, kt, :], start=(kt == 0), stop=(kt == 1))
                    pf = psS.tile([128, 2 * H], F32, tag="s")
                    for kt in range(2):
                        nc.tensor.matmul(pf[:], xt_t[:, kt, :],
                                         wa_sb[:, kt, :], start=(kt == 0), stop=(kt == 1))
                    nc.scalar.copy(F_sb[:, :, it], pf[:])
                    nc.scalar.activation(t_sb[:, :, it], pf[:], AF.Exp,
                                         scale=(1.0 - ALPHA))
                    Gt = wp.tile([128, 2 * H], BF16, tag="Gt")
                    nc.scalar.activation(Gt[:], pf[:], AF.Exp, scale=ALPHA)
                    g2 = Gt.rearrange("p (h two) -> p h two", two=2)[:, :, 1:2]
                    nc.vector.tensor_tensor(
                        XmAll[:, :, it, 0:HID],
                        pwh.rearrange("p (h d) -> p h d", h=H),
                        g2.broadcast_to([128, H, HID]),
                        op=ALU.mult)
                    nc.scalar.copy(XmAll[:, :, it, HID:AUG], g2)
                    for h in WAVE1:
                        q = qp.tile([128, ROWS], BF16, tag="q")
                        nc.vector.scalar_tensor_tensor(
                            q[:], rb[h][:], t_sb[:, 2 * h + 1, it:it + 1],
                            mask_sb[:, it, :], op0=ALU.max, op1=ALU.mult)
                        nc.tensor.matmul(pa[h][:], XmAll[:, h, it, :], q[:],
                                         start=(it == 0), stop=(it == JT - 1))

                xcU = bigp.tile([128, 4, ROWS], F32, tag="xcu")  # un-normalized heads
                rows8x = bigp.tile([1, H, ROWS], F32)    # per-head row sums

                def head_out(pah, h):
                    nc.scalar.copy(
                        xcU[(h % 2) * HID:(h % 2) * HID + HID, h // 2, :], pah[0:HID, :])
                    nc.scalar.copy(rows8x[0:1, h, :], pah[HID:AUG, :])

                for h in WAVE1:
                    head_out(pa[h], h)

                # (tail + partial-Wh2 + AllGather per half; half A overlaps wave 2)
                def half_tail(X):
                    rows4 = bigp.tile([4, ROWS], F32, name=f"rows4_{X}")
                    nc.sync.dma_start(out=rows4[:],
                                      in_=rows8x[0:1, 4 * X:4 * X + 4, :])
                    rr4 = bigp.tile([4, ROWS], F32, name=f"rr4_{X}")
                    nc.vector.reciprocal(rr4[:], rows4[:])
                    for k2 in range(2):
                        k = 2 * X + k2
                        prb = psA.tile([128, ROWS], F32, tag="pa")
                        nc.tensor.matmul(prb[:], sel_sb[0:4, k2, :], rr4[:],
                                         start=True, stop=True)
                        nc.vector.tensor_tensor(hn[:, k, :], xcU[:, k, :], prb[:],
                                                op=ALU.mult)
                    ks = slice(2 * X, 2 * X + 2)
                    nc.vector.tensor_scalar(tm[:, ks, :], hn[:, ks, :], 0.0, None,
                                            op0=ALU.min)
                    nc.scalar.activation(xcU[:, ks, :], tm[:, ks, :], AF.Exp)
                    nc.vector.scalar_tensor_tensor(xcT[:, ks, :], hn[:, ks, :], 0.0,
                                                   xcU[:, ks, :],
                                                   op0=ALU.max, op1=ALU.add)
                    wh2h = cp.tile([128, 4, NC], BF16, name=f"wh2h{X}")
                    for it in range(IT):
                        pw2 = psS.tile([128, NC], F32, tag="s")
                        for kt in (2 * X, 2 * X + 1):
                            nc.tensor.matmul(pw2[:], xcT[:, kt, it * 128:(it + 1) * 128],
                                             wout_sb[:, kt, :],
                                             start=(kt == 2 * X), stop=(kt == 2 * X + 1))
                        if X == 0:
                            nc.vector.scalar_tensor_tensor(
                                wh2h[:, it, :], pw2[:], 0.0, csum_sb[:],
                                op0=ALU.add, op1=ALU.subtract)
                        else:
                            nc.vector.tensor_copy(wh2h[:, it, :], pw2[:])
                    ag_in = dpool.tile([128, 4, NC], BF16, name=f"agi{X}")
                    nc.gpsimd.dma_start(ag_in[:], wh2h[:])
                    ag_out = dpool.tile([NCORES, 128, 4, NC], BF16, name=f"ago{X}")
                    nc.gpsimd.collective_compute(
                        "AllGather", ALU.bypass,
                        replica_groups=[list(range(NCORES))],
                        ins=[ag_in.opt()], outs=[ag_out.opt()],
                    )
                    return ag_out

                xcT = bigp.tile([128, 4, ROWS], F32)
                hn = bigp.tile([128, 4, ROWS], F32, tag="hn4")
                tm = bigp.tile([128, 4, ROWS], F32, tag="tm4")
                ag_outA = half_tail(0)

                # ---------------- wave-2 heads ----------------
                if wave2_a:
                    nAF = cp.tile([128, 2 * H, JT], F32)
                    nc.vector.tensor_scalar(nAF[:], F_sb[:], -ALPHA, None, op0=ALU.mult)
                pa2w = {h: psB.tile([AUG, ROWS], F32, tag="pp", name=f"paw{h}")
                        for h in wave2_m + wave2_a}
                for g in range(JT // 4):
                    for h in wave2_a:
                        j0 = g * 4
                        pt4 = wp2.tile([128, 4, ROWS], BF16, tag="pt")
                        for qq in range(4):
                            jt = j0 + qq
                            et = wp.tile([128, ROWS], F32, tag="et")
                            nc.scalar.activation(et[:], rb[h][:], AF.Prelu,
                                                 bias=F_sb[:, 2 * h + 1, jt:jt + 1],
                                                 alpha=ALPHA)
                            # exp(lrelu(s) - a f2)  -> shares G2-scaled XmAll
                            nc.scalar.activation(pt4[:, qq, :], et[:], AF.Exp,
                                                 bias=nAF[:, 2 * h + 1, jt:jt + 1])
                        pmt = wp2.tile([128, 4, ROWS], BF16, tag="pmt")
                        nc.vector.tensor_tensor(pmt[:], pt4[:],
                                                mask_sb[:, j0:j0 + 4, :], op=ALU.mult)
                        for qq in range(4):
                            jt = j0 + qq
                            nc.tensor.matmul(pa2w[h][:], XmAll[:, h, jt, :],
                                             pmt[:, qq, :],
                                             start=(jt == 0), stop=(jt == JT - 1))
                    for qq in range(4):
                        jt = g * 4 + qq
                        for h in wave2_m:
                            q = qp.tile([128, ROWS], BF16, tag="q")
                            nc.vector.scalar_tensor_tensor(
                                q[:], rb[h][:], t_sb[:, 2 * h + 1, jt:jt + 1],
                                mask_sb[:, jt, :], op0=ALU.max, op1=ALU.mult)
                            nc.tensor.matmul(pa2w[h][:], XmAll[:, h, jt, :], q[:],
                                             start=(jt == 0), stop=(jt == JT - 1))
                for h in wave2_m + wave2_a:
                    head_out(pa2w[h], h)

                ag_outB = half_tail(1)
                # gathered rows: core r, it, p -> global row r*512 + it*128 + p
                agA_sb = bigp.tile([128, NCORES * 4, NC], BF16, tag="hn4")
                agB_sb = bigp.tile([128, NCORES * 4, NC], BF16, tag="xcu")
                for r in range(NCORES):
                    nc.sync.dma_start(out=agA_sb[:, r * 4:(r + 1) * 4, :],
                                      in_=ag_outA[r])
                    nc.sync.dma_start(out=agB_sb[:, r * 4:(r + 1) * 4, :],
                                      in_=ag_outB[r])
                wh2f = cp.tile([128, JT, AUG2], BF16)
                nc.gpsimd.memset(wh2f[:, :, NC:AUG2], 1.0)
                nc.vector.tensor_tensor(wh2f[:, :, 0:NC], agA_sb[:], agB_sb[:],
                                        op=ALU.add)



                # f1 for my rows (layer 2): [1, 512] psum
                pf1o = psS.tile([1, ROWS], F32, tag="s")
                for kt in range(4):
                    nc.tensor.matmul(pf1o[:], woa1_sb[:, kt:kt + 1],
                                     xcT[:, kt, :], start=(kt == 0), stop=(kt == 3))
                R1o_bf = cp.tile([1, ROWS], BF16)
                nc.scalar.activation(R1o_bf[:], pf1o[:], AF.Exp, scale=-(1.0 - ALPHA),
                                     bias=consts_sb[0:1, 2:3])

                # f2 for all nodes (layer 2)
                f2o = cp.tile([128, JT], F32)
                t41b = bigp.tile([128, JT, NC], F32, tag="tm4")
                a2b3 = a2b_sb[:].rearrange("p (o c) -> p o c", o=1)
                nc.vector.tensor_tensor(t41b[:], wh2f[:, :, 0:NC],
                    a2b3.broadcast_to([128, JT, NC]), op=ALU.mult)
                nc.vector.reduce_sum(f2o[:].rearrange("p (k o) -> p k o", o=1),
                                     t41b[:], axis=AX.X)
                t2o = cp.tile([128, JT], F32)
                nc.scalar.activation(t2o[:], f2o[:], AF.Exp, scale=(1.0 - ALPHA))
                G2o = cp.tile([128, JT], F32)
                nc.scalar.activation(G2o[:], f2o[:], AF.Exp, scale=ALPHA)

                # ---------------- layer-2 attention (max-form) ----------------
                rb2i = rbp.tile([128, ROWS], BF16, tag="rb")
                nc.gpsimd.partition_broadcast(rb2i[:], R1o_bf[:])
                Xm2 = hp1.tile([128, JT, AUG2], BF16, tag="Xm2")
                G2o3 = G2o[:].rearrange("p (k o) -> p k o", o=1)
                nc.vector.tensor_tensor(Xm2[:], wh2f[:],
                    G2o3.broadcast_to([128, JT, AUG2]), op=ALU.mult)
                pa2 = psB.tile([AUG2, ROWS], F32, tag="pp")
                for jt in range(JT):
                    q = qp.tile([128, ROWS], BF16, tag="q")
                    nc.vector.scalar_tensor_tensor(
                        q[:], rb2i[:], t2o[:, jt:jt + 1],
                        mask_sb[:, jt, :], op0=ALU.max, op1=ALU.mult)
                    nc.tensor.matmul(pa2[:], Xm2[:, jt, :], q[:],
                                     start=(jt == 0), stop=(jt == JT - 1))
                # normalize + elu'
                hs2 = hp1.tile([AUG2, ROWS], F32, tag="hs2")
                nc.vector.tensor_copy(hs2[:], pa2[:])
                srow2 = hp1.tile([1, ROWS], F32, tag="r1s")
                nc.sync.dma_start(out=srow2[:], in_=hs2[NC:AUG2, :])
                rr2 = hp1.tile([1, ROWS], F32, tag="rr")
                nc.vector.reciprocal(rr2[:], srow2[:])
                rb2 = hp1.tile([128, ROWS], F32, tag="rb2")
                nc.gpsimd.partition_broadcast(rb2[:], rr2[:])
                zn = hp1.tile([NC, ROWS], F32, tag="hn")
                nc.vector.tensor_tensor(zn[:], hs2[0:NC, :], rb2[0:NC, :], op=ALU.mult)
                tm2 = hp1.tile([NC, ROWS], F32, tag="tm")
                nc.vector.tensor_scalar(tm2[:], zn[:], 0.0, None, op0=ALU.min)
                te2 = hp1.tile([NC, ROWS], F32, tag="te")
                nc.scalar.activation(te2[:], tm2[:], AF.Exp)
                zel = hp1.tile([NC, ROWS], F32, tag="tm")
                nc.vector.scalar_tensor_tensor(zel[:], zn[:], 0.0, te2[:],
                                               op0=ALU.max, op1=ALU.add)

                # ---------------- stage 4: log_softmax + out ----------------
                outr = out.rearrange("(t p) c -> p t c", p=128)
                for it in range(IT):
                    ztp = psS.tile([128, NC], F32, tag="s")
                    nc.tensor.transpose(ztp[:], zel[:, it * 128:(it + 1) * 128],
                                        ident_sb[0:NC, 0:NC])
                    zmax = wp.tile([128, 1], F32, tag="zmax")
                    nc.vector.reduce_max(zmax[:], ztp[:], axis=AX.X)
                    nzmax = wp.tile([128, 1], F32, tag="nzmax")
                    nc.vector.tensor_scalar(nzmax[:], zmax[:], -1.0, None, op0=ALU.mult)
                    zsum = wp.tile([128, 1], F32, tag="zsum")
                    zs = wp.tile([128, NC], F32, tag="zs")
                    nc.scalar.activation(zs[:], ztp[:], AF.Exp, bias=nzmax[:],
                                         accum_out=zsum[:])
                    lse = wp.tile([128, 1], F32, tag="lse")
                    nc.scalar.activation(lse[:], zsum[:], AF.Ln)
                    bo = wp.tile([128, 1], F32, tag="bo")
                    nc.vector.scalar_tensor_tensor(bo[:], zmax[:], -1.0, lse[:],
                                                   op0=ALU.mult, op1=ALU.subtract)
                    zf = wp.tile([128, NC], F32, tag="zf")
                    nc.scalar.activation(zf[:], ztp[:], AF.Identity, bias=bo[:])
                    nc.sync.dma_start(out=outr[:, it, :], in_=zf[:])

            for _rep in range(reps):
                body()

    nc.compile()
    return nc


def _host_prep(x, adj, W, a, W_out, a_out):
    bf16 = ml_dtypes.bfloat16
    f32 = np.float32
    x = np.asarray(x, f32)
    W = np.asarray(W, f32)
    a = np.asarray(a, f32)
    W_out = np.asarray(W_out, f32)
    a_out = np.asarray(a_out, f32)

    def pk(arr, kt):  # [kt*128, M] -> [128, kt, M]
        return np.ascontiguousarray(
            arr.reshape(kt, 128, *arr.shape[1:]).transpose(1, 0, *range(2, arr.ndim + 1)))

    xT = pk(np.ascontiguousarray(x.T), 2)                      # [128,2,4096]
    wcat = pk(np.concatenate(list(W), axis=1), 2)              # [128,2,512]
    WA = np.zeros((FIN, 2 * H), f32)
    for h in range(H):
        WA[:, 2 * h] = W[h] @ a[h, :HID]
        WA[:, 2 * h + 1] = W[h] @ a[h, HID:]
    wa = pk(WA, 2)
    wout = pk(W_out, 4)                                        # [128,4,41]
    Woa1 = W_out @ a_out[:NC]                                  # [512]
    woa1 = np.ascontiguousarray(Woa1.reshape(4, 128).T)        # [128,4]
    s = float(Woa1.sum())
    a2b = np.ascontiguousarray(np.broadcast_to(a_out[NC:], (128, NC)))
    csum = np.ascontiguousarray(np.broadcast_to(W_out.sum(0), (128, NC)))
    ident = np.eye(128, NC, dtype=f32)
    consts = np.zeros((128, 8), f32)
    consts[:, 0] = -s
    consts[:, 1] = -ALPHA * s
    consts[:, 2] = (1.0 - ALPHA) * s
    sel = np.zeros((8, 4, 128), f32)
    for h in range(H):
        sel[h, h // 2, (h % 2) * HID:(h % 2) * HID + HID] = 1.0

    shared = dict(xT=xT, wcat=wcat, wa=wa, wout=wout, woa1=woa1, a2b=a2b,
                  csum=csum, ident=ident, consts=consts, sel=sel)
    in_maps = []
    for c in range(NCORES):
        rows = slice(c * ROWS, (c + 1) * ROWS)
        mT = (np.asarray(adj[rows]).T > 0).astype(bf16)        # [4096, 512]
        mT = np.ascontiguousarray(mT.reshape(JT, 128, ROWS).transpose(1, 0, 2))
        xr = pk(np.ascontiguousarray(x[rows].T), 2)            # [128,2,512]
        in_maps.append({**shared, "maskT": mT, "xrT": xr})
    return in_maps


def kernel(x, adj, W, a, W_out, a_out):
    global _CACHED_NC
    if _CACHED_NC is None:
        _CACHED_NC = _build()
    in_maps = _host_prep(x, adj, W, a, W_out, a_out)
    res = run_bass_kernel_spmd(_CACHED_NC, in_maps, list(range(NCORES)))
    out = np.concatenate([res.results[c]["out"] for c in range(NCORES)], axis=0)
    return out.astype(np.float32)


# revision 32
# speedup vs baseline: 1.1449x; 1.1073x over previous
"""GAT (2-layer, 8-head) Bass kernel for 8 Trainium2 NeuronCores.

Sharding: rows (nodes) split 512/core; x and params replicated.
Layer-1 attention per head in transposed layout (partition=j, free=i);
softmax row sums fold into the aggregation matmul via an augmented
column.

Max-form attention: with rho_i = exp(-(1-a) f1_i), t_j = exp((1-a) f2_j):
    m_ij * max(rho_i, t_j) * G2_j = exp(-f1_i) * m_ij * exp(lrelu(f1_i+f2_j))
and the per-row factor exp(-f1_i) cancels in the softmax normalize.
Each head-tile needs ONE fused DVE op q = (rho_b max t_j) * mask
(scalar_tensor_tensor) and ONE matmul against XmAll = G2-scaled Wh_aug
(built during stage 1, fused into the Wh pass; aug column = G2).

Schedule: wave 1 (4 heads) interleaves its STT+matmul stream into the
stage-1 Wh/F loop (PE and DVE both busy); wave 2 runs 3 more max-form
heads plus optionally A-form heads on the otherwise-idle Scalar engine
(exact exp path, rescaled by exp(-a f2) so it shares the same XmAll
stationary operand).  Layer 2 uses the same max-form after a small
AllGather of per-core [512, 41] Wh2 shards.  elu's "-1" is absorbed
algebraically (colsum correction + log_softmax shift invariance).
Row-sum reciprocals are batched into one [8,512] op; the elu tail runs
once on the packed [128,4,512] concat layout with PE-built normalizer
rows.
"""
import sys

sys.path.insert(0, "/opt/trn_rl_repo")

import numpy as np
import ml_dtypes

import concourse.bass as bass
import concourse.bacc as bacc
import concourse.tile as tile
import concourse.mybir as mybir
from concourse.bass_utils import run_bass_kernel_spmd
from concourse import dve_ops as _dvo
from concourse.dve_spec import Spec as _Spec, Src0 as _Src0, Src1 as _Src1, \
    C0 as _C0, maxx as _maxx
from concourse.dve_uop import DveOpSpec as _DveOpSpec


def _register_mask_max():
    """out = max(in0, s0) * in1 as one custom-DVE op (TTSS shape)."""
    name = "MASK_MAX_GAT"
    for op in _dvo.OPS:
        if op.name == name:
            return op
    spec = _Spec(
        body=_maxx(_Src0, _C0) * _Src1,
        reference=lambda in0, in1, s0, s1, imm2:
            (np.maximum(in0.astype(np.float32), s0) * in1).astype(np.float32),
    )
    row = max(_dvo._SUB_OPCODE_FOR_NAME.values()) + 1
    assert row < 0x20
    op = _dvo.DveOp(name, spec, subdim=False, uops_sha={},
                    perf_en={"v3": True, "v4": True})
    _dvo.OPS.append(op)
    _dvo.CUSTOM_DVE_SPECS[name] = spec
    _dvo._SUB_OPCODE_FOR_NAME[name] = row
    for ver in ("v3", "v4"):
        uops = _dvo.lower(spec, ver=ver)
        s = _DveOpSpec(name=name, opcode=row, uops=uops,
                       rd1_en=_dvo.has_src1(spec))
        op.uops_sha[ver] = s.sha(ver)
    return op


_MASK_MAX = _register_mask_max()

F32 = mybir.dt.float32
BF16 = mybir.dt.bfloat16
AF = mybir.ActivationFunctionType
ALU = mybir.AluOpType
AX = mybir.AxisListType

NCORES = 8
N = 4096
FIN = 256
HID = 64
H = 8
NC = 41          # classes
ROWS = N // NCORES   # 512 rows per core
JT = N // 128        # 32 j tiles
IT = ROWS // 128     # 4 i tiles of my rows
AUG = HID + 1        # 65
AUG2 = NC + 1        # 42
ALPHA = 0.2

WAVE1 = (0, 1, 2, 3)
A_COUNT = 1          # heads of wave 2 on the Scalar (exp) path

_CACHED_NC = None


def _build(trace_sim=False, reps=1, ablate=()):
    nc = bacc.Bacc("TRN2", target_bir_lowering=False, debug=False,
                   num_devices=NCORES)
    d = {}
    def dram_in(name, shape, dt=F32):
        d[name] = nc.dram_tensor(name, list(shape), dt, kind="ExternalInput").ap()
        return d[name]

    xT = dram_in("xT", [128, 2, N], BF16)
    xrT = dram_in("xrT", [128, 2, ROWS], BF16)
    wcat = dram_in("wcat", [128, 2, H * HID], BF16)
    wa = dram_in("wa", [128, 2, 2 * H], BF16)
    wout = dram_in("wout", [128, 4, NC])
    woa1 = dram_in("woa1", [128, 4])
    a2b = dram_in("a2b", [128, NC])
    csum = dram_in("csum", [128, NC])
    ident = dram_in("ident", [128, NC])
    consts = dram_in("consts", [128, 8])
    sel = dram_in("sel", [8, 4, 128])
    maskT = dram_in("maskT", [128, JT, ROWS], BF16)
    out = nc.dram_tensor("out", [ROWS, NC], F32, kind="ExternalOutput").ap()

    a_count = A_COUNT
    if "A0" in ablate:
        a_count = 0
    if "A2" in ablate:
        a_count = 2
    wave2_m = tuple(range(4, 8 - a_count))
    wave2_a = tuple(range(8 - a_count, 8))

    with tile.TileContext(nc, trace_sim=trace_sim) as tc:
        with (
            tc.tile_pool(name="dram", bufs=1, space="DRAM") as dpool,
            tc.tile_pool(name="const", bufs=1) as cp,
            tc.tile_pool(name="big", bufs=1) as bigp,
            tc.tile_pool(name="rbp", bufs=8) as rbp,
            tc.tile_pool(name="qp", bufs=6) as qp,
            tc.tile_pool(name="work", bufs=4) as wp,
            tc.tile_pool(name="work2", bufs=2) as wp2,
            tc.tile_pool(name="head1", bufs=1) as hp1,
            tc.tile_pool(name="psA", bufs=2, space="PSUM") as psA,
            tc.tile_pool(name="psB", bufs=4, space="PSUM") as psB,
            tc.tile_pool(name="psS", bufs=2, space="PSUM") as psS,
        ):
            # ---------------- stage 0: loads ----------------
            xrT_sb = cp.tile([128, 2, ROWS], BF16)
            nc.sync.dma_start(out=xrT_sb[:], in_=xrT[:])
            wcat_sb = cp.tile([128, 2, H * HID], BF16)
            nc.sync.dma_start(out=wcat_sb[:], in_=wcat[:])
            wa_sb = cp.tile([128, 2, 2 * H], BF16)
            nc.sync.dma_start(out=wa_sb[:], in_=wa[:])
            wout_sb = cp.tile([128, 4, NC], F32)
            nc.sync.dma_start(out=wout_sb[:], in_=wout[:])
            woa1_sb = cp.tile([128, 4], F32)
            nc.sync.dma_start(out=woa1_sb[:], in_=woa1[:])
            a2b_sb = cp.tile([128, NC], F32)
            nc.sync.dma_start(out=a2b_sb[:], in_=a2b[:])
            csum_sb = cp.tile([128, NC], F32)
            nc.sync.dma_start(out=csum_sb[:], in_=csum[:])
            ident_sb = cp.tile([128, NC], F32)
            nc.sync.dma_start(out=ident_sb[:], in_=ident[:])
            consts_sb = cp.tile([128, 8], F32)
            nc.sync.dma_start(out=consts_sb[:], in_=consts[:])
            sel_sb = cp.tile([8, 4, 128], F32)
            nc.sync.dma_start(out=sel_sb[:], in_=sel[:])
            mask_sb = bigp.tile([128, JT, ROWS], BF16)
            for mc in range(4):
                nc.scalar.dma_start(out=mask_sb[:, mc * 8:(mc + 1) * 8, :],
                                   in_=maskT[:, mc * 8:(mc + 1) * 8, :])

            def body():
                # ---------------- prologue: rho for my rows ----------------
                pfmy = psS.tile([2 * H, ROWS], F32, tag="s")
                for kt in range(2):
                    nc.tensor.matmul(pfmy[:], wa_sb[:, kt, :], xrT_sb[:, kt, :],
                                     start=(kt == 0), stop=(kt == 1))
                rho_bf = cp.tile([2 * H, ROWS], BF16)
                nc.scalar.activation(rho_bf[:], pfmy[:], AF.Exp, scale=-(1.0 - ALPHA))
                fmy_bf = cp.tile([2 * H, ROWS], BF16)
                nc.scalar.copy(fmy_bf[:], pfmy[:])
                # per-head broadcasts: rho for max-form heads, f1 for A-form
                rb = {}
                for h in range(H):
                    src = fmy_bf if h in wave2_a else rho_bf
                    rs = hp1.tile([1, ROWS], BF16, tag="f1s")
                    nc.sync.dma_start(out=rs[:], in_=src[2 * h:2 * h + 1, :])
                    rb[h] = rbp.tile([128, ROWS], BF16, tag="rb", name=f"rb{h}")
                    nc.gpsimd.partition_broadcast(rb[h][:], rs[:])

                # ---------------- stage 1 + wave-1 heads ----------------
                XmAll = bigp.tile([128, H, JT, AUG], BF16)
                F_sb = cp.tile([128, 2 * H, JT], F32)
                t_sb = cp.tile([128, 2 * H, JT], F32)
                pa = {h: psB.tile([AUG, ROWS], F32, tag="pp", name=f"pa{h}")
                      for h in WAVE1}
                for it in range(JT):
                    xt_t = wp.tile([128, 2, 128], BF16, tag="xt")
                    nc.sync.dma_start(out=xt_t[:], in_=xT[:, :, it * 128:(it + 1) * 128])
                    pwh = psA.tile([128, H * HID], F32, tag="pa")
                    for kt in range(2):
                        nc.tensor.matmul(pwh[:], xt_t[:, kt, :],
                                         wcat_sb[:, kt, :], start=(kt == 0), stop=(kt == 1))
                    pf = psS.tile([128, 2 * H], F32, tag="s")
                    for kt in range(2):
                        nc.tensor.matmul(pf[:], xt_t[:, kt, :],
                                         wa_sb[:, kt, :], start=(kt == 0), stop=(kt == 1))
                    nc.scalar.copy(F_sb[:, :, it], pf[:])
                    nc.scalar.activation(t_sb[:, :, it], pf[:], AF.Exp,
                                         scale=(1.0 - ALPHA))
                    Gt = wp.tile([128, 2 * H], BF16, tag="Gt")
                    nc.scalar.activation(Gt[:], pf[:], AF.Exp, scale=ALPHA)
                    g2 = Gt.rearrange("p (h two) -> p h two", two=2)[:, :, 1:2]
                    nc.vector.tensor_tensor(
                        XmAll[:, :, it, 0:HID],
                        pwh.rearrange("p (h d) -> p h d", h=H),
                        g2.broadcast_to([128, H, HID]),
                        op=ALU.mult)
                    nc.scalar.copy(XmAll[:, :, it, HID:AUG], g2)
                    for h in WAVE1:
                        q = qp.tile([128, ROWS], BF16, tag="q")
                        nc.vector._custom_dve(
                            _MASK_MAX, out=q[:], in0=rb[h][:],
                            in1=mask_sb[:, it, :],
                            s0=t_sb[:, 2 * h + 1, it:it + 1])
                        nc.tensor.matmul(pa[h][:], XmAll[:, h, it, :], q[:],
                                         start=(it == 0), stop=(it == JT - 1))

                xcU = bigp.tile([128, 4, ROWS], F32, tag="xcu")  # un-normalized heads
                rows8x = bigp.tile([1, H, ROWS], F32)    # per-head row sums

                def head_out(pah, h):
                    nc.scalar.copy(
                        xcU[(h % 2) * HID:(h % 2) * HID + HID, h // 2, :], pah[0:HID, :])
                    nc.scalar.copy(rows8x[0:1, h, :], pah[HID:AUG, :])

                for h in WAVE1:
                    head_out(pa[h], h)

                # (tail + partial-Wh2 + AllGather per half; half A overlaps wave 2)
                def half_tail(X):
                    rows4 = bigp.tile([4, ROWS], F32, name=f"rows4_{X}")
                    nc.sync.dma_start(out=rows4[:],
                                      in_=rows8x[0:1, 4 * X:4 * X + 4, :])
                    rr4 = bigp.tile([4, ROWS], F32, name=f"rr4_{X}")
                    nc.vector.reciprocal_approx_fast(rr4[:], rows4[:])
                    for k2 in range(2):
                        k = 2 * X + k2
                        prb = psA.tile([128, ROWS], F32, tag="pa")
                        nc.tensor.matmul(prb[:], sel_sb[0:4, k2, :], rr4[:],
                                         start=True, stop=True)
                        nc.vector.tensor_tensor(hn[:, k, :], xcU[:, k, :], prb[:],
                                                op=ALU.mult)
                    ks = slice(2 * X, 2 * X + 2)
                    nc.vector.tensor_scalar(tm[:, ks, :], hn[:, ks, :], 0.0, None,
                                            op0=ALU.min)
                    nc.scalar.activation(xcU[:, ks, :], tm[:, ks, :], AF.Exp)
                    nc.vector.scalar_tensor_tensor(xcT[:, ks, :], hn[:, ks, :], 0.0,
                                                   xcU[:, ks, :],
                                                   op0=ALU.max, op1=ALU.add)
                    wh2h = cp.tile([128, 4, NC], BF16, name=f"wh2h{X}")
                    for it in range(IT):
                        pw2 = psS.tile([128, NC], F32, tag="s")
                        for kt in (2 * X, 2 * X + 1):
                            nc.tensor.matmul(pw2[:], xcT[:, kt, it * 128:(it + 1) * 128],
                                             wout_sb[:, kt, :],
                                             start=(kt == 2 * X), stop=(kt == 2 * X + 1))
                        if X == 0:
                            nc.vector.scalar_tensor_tensor(
                                wh2h[:, it, :], pw2[:], 0.0, csum_sb[:],
                                op0=ALU.add, op1=ALU.subtract)
                        else:
                            nc.vector.tensor_copy(wh2h[:, it, :], pw2[:])
                    ag_in = dpool.tile([128, 4, NC], BF16, name=f"agi{X}")
                    nc.gpsimd.dma_start(ag_in[:], wh2h[:])
                    ag_out = dpool.tile([NCORES, 128, 4, NC], BF16, name=f"ago{X}")
                    nc.gpsimd.collective_compute(
                        "AllGather", ALU.bypass,
                        replica_groups=[list(range(NCORES))],
                        ins=[ag_in.opt()], outs=[ag_out.opt()],
                    )
                    return ag_out

                xcT = bigp.tile([128, 4, ROWS], F32)
                hn = bigp.tile([128, 4, ROWS], F32, tag="hn4")
                tm = bigp.tile([128, 4, ROWS], F32, tag="tm4")
                ag_outA = half_tail(0)

                # ---------------- wave-2 heads ----------------
                if wave2_a:
                    nAF = cp.tile([128, 2 * H, JT], F32)
                    nc.vector.tensor_scalar(nAF[:], F_sb[:], -ALPHA, None, op0=ALU.mult)
                pa2w = {h: psB.tile([AUG, ROWS], F32, tag="pp", name=f"paw{h}")
                        for h in wave2_m + wave2_a}
                for g in range(JT // 4):
                    for h in wave2_a:
                        j0 = g * 4
                        pt4 = wp2.tile([128, 4, ROWS], BF16, tag="pt")
                        for qq in range(4):
                            jt = j0 + qq
                            et = wp.tile([128, ROWS], F32, tag="et")
                            nc.scalar.activation(et[:], rb[h][:], AF.Prelu,
                                                 bias=F_sb[:, 2 * h + 1, jt:jt + 1],
                                                 alpha=ALPHA)
                            # exp(lrelu(s) - a f2)  -> shares G2-scaled XmAll
                            nc.scalar.activation(pt4[:, qq, :], et[:], AF.Exp,
                                                 bias=nAF[:, 2 * h + 1, jt:jt + 1])
                        pmt = wp2.tile([128, 4, ROWS], BF16, tag="pmt")
                        nc.vector.tensor_tensor(pmt[:], pt4[:],
                                                mask_sb[:, j0:j0 + 4, :], op=ALU.mult)
                        for qq in range(4):
                            jt = j0 + qq
                            nc.tensor.matmul(pa2w[h][:], XmAll[:, h, jt, :],
                                             pmt[:, qq, :],
                                             start=(jt == 0), stop=(jt == JT - 1))
                    for qq in range(4):
                        jt = g * 4 + qq
                        for h in wave2_m:
                            q = qp.tile([128, ROWS], BF16, tag="q")
                            nc.vector._custom_dve(
                                _MASK_MAX, out=q[:], in0=rb[h][:],
                                in1=mask_sb[:, jt, :],
                                s0=t_sb[:, 2 * h + 1, jt:jt + 1])
                            nc.tensor.matmul(pa2w[h][:], XmAll[:, h, jt, :], q[:],
                                             start=(jt == 0), stop=(jt == JT - 1))
                for h in wave2_m + wave2_a:
                    head_out(pa2w[h], h)

                ag_outB = half_tail(1)
                # gathered rows: core r, it, p -> global row r*512 + it*128 + p
                agA_sb = bigp.tile([128, NCORES * 4, NC], BF16, tag="hn4")
                agB_sb = bigp.tile([128, NCORES * 4, NC], BF16, tag="xcu")
                for r in range(NCORES):
                    nc.sync.dma_start(out=agA_sb[:, r * 4:(r + 1) * 4, :],
                                      in_=ag_outA[r])
                    nc.sync.dma_start(out=agB_sb[:, r * 4:(r + 1) * 4, :],
                                      in_=ag_outB[r])
                wh2f = cp.tile([128, JT, AUG2], BF16)
                nc.gpsimd.memset(wh2f[:, :, NC:AUG2], 1.0)
                nc.vector.tensor_tensor(wh2f[:, :, 0:NC], agA_sb[:], agB_sb[:],
                                        op=ALU.add)



                # f1 for my rows (layer 2): [1, 512] psum
                pf1o = psS.tile([1, ROWS], F32, tag="s")
                for kt in range(4):
                    nc.tensor.matmul(pf1o[:], woa1_sb[:, kt:kt + 1],
                                     xcT[:, kt, :], start=(kt == 0), stop=(kt == 3))
                R1o_bf = cp.tile([1, ROWS], BF16)
                nc.scalar.activation(R1o_bf[:], pf1o[:], AF.Exp, scale=-(1.0 - ALPHA),
                                     bias=consts_sb[0:1, 2:3])

                # f2 for all nodes (layer 2)
                f2o = cp.tile([128, JT], F32)
                t41b = bigp.tile([128, JT, NC], F32, tag="tm4")
                a2b3 = a2b_sb[:].rearrange("p (o c) -> p o c", o=1)
                nc.vector.tensor_tensor(t41b[:], wh2f[:, :, 0:NC],
                    a2b3.broadcast_to([128, JT, NC]), op=ALU.mult)
                nc.vector.reduce_sum(f2o[:].rearrange("p (k o) -> p k o", o=1),
                                     t41b[:], axis=AX.X)
                t2o = cp.tile([128, JT], F32)
                nc.scalar.activation(t2o[:], f2o[:], AF.Exp, scale=(1.0 - ALPHA))
                G2o = cp.tile([128, JT], F32)
                nc.scalar.activation(G2o[:], f2o[:], AF.Exp, scale=ALPHA)

                # ---------------- layer-2 attention (max-form) ----------------
                rb2i = rbp.tile([128, ROWS], BF16, tag="rb")
                nc.gpsimd.partition_broadcast(rb2i[:], R1o_bf[:])
                Xm2 = hp1.tile([128, JT, AUG2], BF16, tag="Xm2")
                G2o3 = G2o[:].rearrange("p (k o) -> p k o", o=1)
                nc.vector.tensor_tensor(Xm2[:], wh2f[:],
                    G2o3.broadcast_to([128, JT, AUG2]), op=ALU.mult)
                pa2 = psB.tile([AUG2, ROWS], F32, tag="pp")
                for jt in range(JT):
                    q = qp.tile([128, ROWS], BF16, tag="q")
                    nc.vector._custom_dve(
                        _MASK_MAX, out=q[:], in0=rb2i[:],
                        in1=mask_sb[:, jt, :], s0=t2o[:, jt:jt + 1])
                    nc.tensor.matmul(pa2[:], Xm2[:, jt, :], q[:],
                                     start=(jt == 0), stop=(jt == JT - 1))
                # normalize + elu'
                hs2 = hp1.tile([AUG2, ROWS], F32, tag="hs2")
                nc.vector.tensor_copy(hs2[:], pa2[:])
                srow2 = hp1.tile([1, ROWS], F32, tag="r1s")
                nc.sync.dma_start(out=srow2[:], in_=hs2[NC:AUG2, :])
                rr2 = hp1.tile([1, ROWS], F32, tag="rr")
                nc.vector.reciprocal_approx_fast(rr2[:], srow2[:])
                rb2 = hp1.tile([128, ROWS], F32, tag="rb2")
                nc.gpsimd.partition_broadcast(rb2[:], rr2[:])
                zn = hp1.tile([NC, ROWS], F32, tag="hn")
                nc.vector.tensor_tensor(zn[:], hs2[0:NC, :], rb2[0:NC, :], op=ALU.mult)
                tm2 = hp1.tile([NC, ROWS], F32, tag="tm")
                nc.vector.tensor_scalar(tm2[:], zn[:], 0.0, None, op0=ALU.min)
                te2 = hp1.tile([NC, ROWS], F32, tag="te")
                nc.scalar.activation(te2[:], tm2[:], AF.Exp)
                zel = hp1.tile([NC, ROWS], F32, tag="tm")
                nc.vector.scalar_tensor_tensor(zel[:], zn[:], 0.0, te2[:],
                                               op0=ALU.max, op1=ALU.add)

                # ---------------- stage 4: log_softmax + out ----------------
                outr = out.rearrange("(t p) c -> p t c", p=128)
                for it in range(IT):
                    ztp = psS.tile([128, NC], F32, tag="s")
                    nc.tensor.transpose(ztp[:], zel[:, it * 128:(it + 1) * 128],
                                        ident_sb[0:NC, 0:NC])
                    zmax = wp.tile([128, 1], F32, tag="zmax")
                    nc.vector.reduce_max(zmax[:], ztp[:], axis=AX.X)
                    nzmax = wp.tile([128, 1], F32, tag="nzmax")
                    nc.vector.tensor_scalar(nzmax[:], zmax[:], -1.0, None, op0=ALU.mult)
                    zsum = wp.tile([128, 1], F32, tag="zsum")
                    zs = wp.tile([128, NC], F32, tag="zs")
                    nc.scalar.activation(zs[:], ztp[:], AF.Exp, bias=nzmax[:],
                                         accum_out=zsum[:])
                    lse = wp.tile([128, 1], F32, tag="lse")
                    nc.scalar.activation(lse[:], zsum[:], AF.Ln)
                    bo = wp.tile([128, 1], F32, tag="bo")
                    nc.vector.scalar_tensor_tensor(bo[:], zmax[:], -1.0, lse[:],
                                                   op0=ALU.mult, op1=ALU.subtract)
                    zf = wp.tile([128, NC], F32, tag="zf")
                    nc.scalar.activation(zf[:], ztp[:], AF.Identity, bias=bo[:])
                    nc.sync.dma_start(out=outr[:, it, :], in_=zf[:])

            for _rep in range(reps):
                body()

    nc.compile()
    return nc


def _host_prep(x, adj, W, a, W_out, a_out):
    bf16 = ml_dtypes.bfloat16
    f32 = np.float32
    x = np.asarray(x, f32)
    W = np.asarray(W, f32)
    a = np.asarray(a, f32)
    W_out = np.asarray(W_out, f32)
    a_out = np.asarray(a_out, f32)

    def pk(arr, kt):  # [kt*128, M] -> [128, kt, M]
        return np.ascontiguousarray(
            arr.reshape(kt, 128, *arr.shape[1:]).transpose(1, 0, *range(2, arr.ndim + 1)))

    xT = pk(np.ascontiguousarray(x.T), 2).astype(bf16)         # [128,2,4096]
    wcat = pk(np.concatenate(list(W), axis=1), 2).astype(bf16)  # [128,2,512]
    WA = np.zeros((FIN, 2 * H), f32)
    for h in range(H):
        WA[:, 2 * h] = W[h] @ a[h, :HID]
        WA[:, 2 * h + 1] = W[h] @ a[h, HID:]
    wa = pk(WA, 2).astype(bf16)
    wout = pk(W_out, 4)                                        # [128,4,41]
    Woa1 = W_out @ a_out[:NC]                                  # [512]
    woa1 = np.ascontiguousarray(Woa1.reshape(4, 128).T)        # [128,4]
    s = float(Woa1.sum())
    a2b = np.ascontiguousarray(np.broadcast_to(a_out[NC:], (128, NC)))
    csum = np.ascontiguousarray(np.broadcast_to(W_out.sum(0), (128, NC)))
    ident = np.eye(128, NC, dtype=f32)
    consts = np.zeros((128, 8), f32)
    consts[:, 0] = -s
    consts[:, 1] = -ALPHA * s
    consts[:, 2] = (1.0 - ALPHA) * s
    sel = np.zeros((8, 4, 128), f32)
    for h in range(H):
        sel[h, h // 2, (h % 2) * HID:(h % 2) * HID + HID] = 1.0

    shared = dict(xT=xT, wcat=wcat, wa=wa, wout=wout, woa1=woa1, a2b=a2b,
                  csum=csum, ident=ident, consts=consts, sel=sel)
    in_maps = []
    for c in range(NCORES):
        rows = slice(c * ROWS, (c + 1) * ROWS)
        mT = (np.asarray(adj[rows]).T > 0).astype(bf16)        # [4096, 512]
        mT = np.ascontiguousarray(mT.reshape(JT, 128, ROWS).transpose(1, 0, 2))
        xr = pk(np.ascontiguousarray(x[rows].T), 2).astype(bf16)  # [128,2,512]
        in_maps.append({**shared, "maskT": mT, "xrT": xr})
    return in_maps


def kernel(x, adj, W, a, W_out, a_out):
    global _CACHED_NC
    if _CACHED_NC is None:
        _CACHED_NC = _build()
    in_maps = _host_prep(x, adj, W, a, W_out, a_out)
    res = run_bass_kernel_spmd(_CACHED_NC, in_maps, list(range(NCORES)))
    out = np.concatenate([res.results[c]["out"] for c in range(NCORES)], axis=0)
    return out.astype(np.float32)
